# revision 8
# baseline (speedup 1.0000x reference)
"""ChemicalGraphConv GNN message-passing kernel for 8 Trainium2 NeuronCores.

Strategy (destination sharding, no collectives):
  - Nodes are split into 8 equal contiguous ranges (6250/core). Each core
    handles the edges whose destination (row) falls in its range and
    produces that slice of the output. No all-reduce needed.
  - Math factorization:  msg_e = relu(h[row]@W1a + h[col]@W1b + bond@W1c + b1)
    gated by g_e = sigmoid(bond@Wa + ba), then @W2 + b2, scatter-add on row.
    Since g_e > 0 and the final @W2 is linear:
       out_r = (sum_e S[e,r] * g_e * relu(t_e)) @ W2 + (sum_e S[e,r] g_e) * b2
    so W2 is applied per 128-row window, not per edge.
  - Per core: precompute A = h_slice@W1a (6272x128) and B = h@W1b (50176x128)
    in bf16 DRAM tables; per edge DMA-gather A[row_local] and B[col].
  - dma_gather indices are int16, so the col gather is done in two passes
    (col < 32768 from B[:32768], col >= 32768 from B[32768:]).
  - Edges are grouped into 49 windows of 128 destination rows; a gated
    one-hot matrix S (built by one DVE tensor_scalar op from iota/rowidx/g)
    does the segment reduction as a matmul accumulating in PSUM.
"""

import math
from dataclasses import dataclass

import ml_dtypes
import numpy as np

import concourse.bass as bass
import concourse.mybir as mybir
import concourse.tile as tile
from concourse import bacc, library_config
from concourse.bass import ds, ts
from concourse.bass_utils import run_bass_kernel_spmd

BF16 = mybir.dt.bfloat16
F32 = mybir.dt.float32
I16 = mybir.dt.int16
NPBF16 = ml_dtypes.bfloat16

AF = mybir.ActivationFunctionType
ALU = mybir.AluOpType


@dataclass(frozen=True)
class Cfg:
    n: int = 50000          # nodes
    d: int = 128            # feature dim
    bond: int = 64          # bond feature dim
    cores: int = 8
    lo_limit: int = 32768   # int16 gather index split point
    sb: int = 1024          # gather superblock (edges)

    @property
    def npc(self):          # nodes per core (destination range)
        return self.n // self.cores

    @property
    def win(self):          # 128-row windows per core
        return math.ceil(self.npc / 128)

    @property
    def npad_core(self):
        return self.win * 128

    @property
    def npad_glob(self):    # padded global node count (B table rows)
        return ((self.n + 127) // 128) * 128

    @property
    def tps(self):          # tiles (of 128 edges) per superblock
        return self.sb // 128


FULL = Cfg()


# --------------------------------------------------------------------------
# Host-side preprocessing
# --------------------------------------------------------------------------

def _pack16(v):
    # edge i -> [i % 16, i // 16], replicated to 128 partitions (8 q7 cores)
    return np.ascontiguousarray(np.tile(v.reshape(-1, 16).T, (8, 1)))


def _pack128(v):
    # edge i -> [i % 128, i // 128]
    return np.ascontiguousarray(v.reshape(-1, 128).T)


def preprocess(h, edge_index, bond_emb, W1, b1, W2, b2, Wa, ba, cfg=FULL):
    """Returns (in_maps, meta) for the 8 cores."""
    c = cfg
    h = np.asarray(h, np.float32)
    row = np.asarray(edge_index[0]).astype(np.int64)
    col = np.asarray(edge_index[1]).astype(np.int64)
    bond = np.asarray(bond_emb, np.float32)
    W1 = np.asarray(W1, np.float32)
    W2m = np.asarray(W2, np.float32)
    b1 = np.asarray(b1, np.float32)
    b2 = np.asarray(b2, np.float32)
    Wa = np.asarray(Wa, np.float32).reshape(c.bond, 1)
    ba = np.asarray(ba, np.float32).reshape(1)

    core_of = row // c.npc

    # First sweep: per (core, pass) window counts -> global tile budgets
    percore = []
    tw = {"lo": 1, "hi": 1}
    for k in range(c.cores):
        m = core_of == k
        r = (row[m] - k * c.npc).astype(np.int64)
        cc = col[m]
        bb = bond[m]
        lo = cc < c.lo_limit
        entry = {}
        for pname, pm in (("lo", lo), ("hi", ~lo)):
            rp, cp, bp = r[pm], cc[pm], bb[pm]
            w = rp >> 7
            order = np.argsort(w, kind="stable")
            rp, cp, bp, w = rp[order], cp[order], bp[order], w[order]
            counts = np.bincount(w, minlength=c.win).astype(np.int64)
            entry[pname] = (rp, cp, bp, counts)
            if len(rp):
                tw[pname] = max(tw[pname], int(math.ceil(counts.max() / 128)))
        percore.append(entry)

    def stream_len(twp):
        n_tiles = c.win * twp
        n_sb = math.ceil(n_tiles / c.tps)
        return n_sb * c.sb

    L = {p: stream_len(tw[p]) for p in ("lo", "hi")}

    # Constants (shared across cores)
    hT = np.zeros((c.d, c.npad_glob), np.float32)
    hT[:, : c.n] = h.T
    consts = {
        "hT": hT.astype(NPBF16),
        "W1a": np.ascontiguousarray(W1[: c.d]).astype(NPBF16),
        "W1b": np.ascontiguousarray(W1[c.d : 2 * c.d]).astype(NPBF16),
        "W1c": np.vstack([W1[2 * c.d :], b1[None, :]]).astype(NPBF16),
        "Wa": np.vstack([Wa, ba[None, :]]).astype(NPBF16),
        "W2": W2m.astype(NPBF16),
        "b2rep": np.tile(b2[None, :], (128, 1)).astype(np.float32),
        "iden": np.eye(128, dtype=np.float32).astype(NPBF16),
        "iotaT": np.tile(np.arange(128, dtype=np.float32)[None, :], (128, 1)).astype(
            NPBF16
        ),
        "ones": np.ones((128, 1), np.float32).astype(NPBF16),
    }

    in_maps = []
    for k in range(c.cores):
        im = dict(consts)

        # core's h slice, transposed (for A) and window-major f32 (residual)
        hsl = np.zeros((c.npad_core, c.d), np.float32)
        nvalid = min(c.npc, c.n - k * c.npc)
        hsl[:nvalid] = h[k * c.npc : k * c.npc + nvalid]
        im["hTs"] = np.ascontiguousarray(hsl.T).astype(NPBF16)
        im["hs"] = np.ascontiguousarray(
            hsl.reshape(c.win, 128, c.d).transpose(1, 0, 2)
        )

        for pname in ("lo", "hi"):
            rp, cp, bp, counts = percore[k][pname]
            Lp = L[pname]
            twp = tw[pname]
            colv = np.zeros(Lp, np.int64)
            growv = np.zeros(Lp, np.int64)
            rlocv = np.full(Lp, 999.0, np.float32)
            bT = np.zeros((c.bond + 1, Lp), np.float32)
            bT[c.bond, :] = 1.0
            starts = np.zeros(c.win, np.int64)
            starts[1:] = np.cumsum(counts)[:-1]
            w = rp >> 7
            pos = w * (twp * 128) + (np.arange(len(rp)) - starts[w])
            if pname == "hi":
                colv[pos] = cp - c.lo_limit
            else:
                colv[pos] = cp
            growv[pos] = rp
            rlocv[pos] = (rp & 127).astype(np.float32)
            bT[: c.bond, pos] = bp.T
            im[f"col_{pname}"] = _pack16(colv.astype(np.int16))
            im[f"grow_{pname}"] = _pack16(growv.astype(np.int16))
            im[f"rloc_{pname}"] = _pack128(rlocv)
            im[f"bondT_{pname}"] = bT.astype(NPBF16)
        in_maps.append(im)

    meta = {"tw_lo": tw["lo"], "tw_hi": tw["hi"], "L_lo": L["lo"], "L_hi": L["hi"]}
    return in_maps, meta


# --------------------------------------------------------------------------
# Device program
# --------------------------------------------------------------------------

def build(meta, cfg=FULL):
    c = cfg
    nc = bacc.Bacc("TRN2", target_bir_lowering=False, debug=False)

    def din(name, shape, dt):
        return nc.dram_tensor(name, list(shape), dt, kind="ExternalInput")

    hT_d = din("hT", (c.d, c.npad_glob), BF16)
    hTs_d = din("hTs", (c.d, c.npad_core), BF16)
    hs_d = din("hs", (128, c.win, c.d), F32)
    W1a_d = din("W1a", (c.d, c.d), BF16)
    W1b_d = din("W1b", (c.d, c.d), BF16)
    W1c_d = din("W1c", (c.bond + 1, c.d), BF16)
    Wa_d = din("Wa", (c.bond + 1, 1), BF16)
    W2_d = din("W2", (c.d, c.d), BF16)
    b2rep_d = din("b2rep", (128, c.d), F32)
    iden_d = din("iden", (128, 128), BF16)
    iotaT_d = din("iotaT", (128, 128), BF16)
    ones_d = din("ones", (128, 1), BF16)

    passes = []
    for pname in ("lo", "hi"):
        Lp = meta[f"L_{pname}"]
        passes.append(
            dict(
                name=pname,
                tw=meta[f"tw_{pname}"],
                L=Lp,
                bondT=din(f"bondT_{pname}", (c.bond + 1, Lp), BF16),
                col=din(f"col_{pname}", (128, Lp // 16), I16),
                grow=din(f"grow_{pname}", (128, Lp // 16), I16),
                rloc=din(f"rloc_{pname}", (128, Lp // 128), F32),
            )
        )

    out_d = nc.dram_tensor("out", [128, c.win, c.d], F32, kind="ExternalOutput")
    A_d = nc.dram_tensor("A_tab", [c.npad_core, c.d], BF16)
    B_d = nc.dram_tensor("B_tab", [c.npad_glob, c.d], BF16)

    with tile.TileContext(nc) as tc:
        nc.gpsimd.load_library(library_config.mlp)
        with tc.tile_pool(name="const", bufs=1) as cpool:
            W1a_sb = cpool.tile([c.d, c.d], BF16)
            W1b_sb = cpool.tile([c.d, c.d], BF16)
            W1c_sb = cpool.tile([c.bond + 1, c.d], BF16)
            Wa_sb = cpool.tile([c.bond + 1, 1], BF16)
            W2_sb = cpool.tile([c.d, c.d], BF16)
            b2rep_sb = cpool.tile([128, c.d], F32)
            iden_sb = cpool.tile([128, 128], BF16)
            iotaT_sb = cpool.tile([128, 128], BF16)
            ones_sb = cpool.tile([128, 1], BF16)
            for sb_t, dr in (
                (W1a_sb, W1a_d), (W1b_sb, W1b_d), (W1c_sb, W1c_d),
                (Wa_sb, Wa_d), (W2_sb, W2_d), (b2rep_sb, b2rep_d),
                (iden_sb, iden_d), (iotaT_sb, iotaT_d), (ones_sb, ones_d),
            ):
                nc.sync.dma_start(out=sb_t[:], in_=dr[:])

            h1T_sb = cpool.tile([128, c.win * 128], BF16)
            degg_sb = cpool.tile([128, c.win], F32)

            # ---------------- Phase 1: A and B tables ----------------
            with tc.tile_pool(name="ph1", bufs=4) as p1, \
                 tc.tile_pool(name="ph1p", bufs=4, space="PSUM") as p1p:
                for nt in range(c.npad_glob // 128):
                    hT_t = p1.tile([c.d, 128], BF16, tag="ht")
                    nc.sync.dma_start(out=hT_t[:], in_=hT_d[:, ts(nt, 128)])
                    ps = p1p.tile([128, c.d], F32)
                    nc.tensor.matmul(ps[:], lhsT=hT_t[:], rhs=W1b_sb[:],
                                     start=True, stop=True)
                    ob = p1.tile([128, c.d], BF16, tag="ob")
                    nc.scalar.activation(ob[:], ps[:], AF.Copy)
                    nc.sync.dma_start(out=B_d[ts(nt, 128), :], in_=ob[:])
                for w in range(c.win):
                    hT_t = p1.tile([c.d, 128], BF16, tag="ht")
                    nc.sync.dma_start(out=hT_t[:], in_=hTs_d[:, ts(w, 128)])
                    ps = p1p.tile([128, c.d], F32)
                    nc.tensor.matmul(ps[:], lhsT=hT_t[:], rhs=W1a_sb[:],
                                     start=True, stop=True)
                    ob = p1.tile([128, c.d], BF16, tag="ob")
                    nc.scalar.activation(ob[:], ps[:], AF.Copy)
                    nc.sync.dma_start(out=A_d[ts(w, 128), :], in_=ob[:])

            # ---------------- Phase 2: edge passes ----------------
            with tc.tile_pool(name="gsb", bufs=2) as pg, \
                 tc.tile_pool(name="gpsum", bufs=2, space="PSUM") as pgp, \
                 tc.tile_pool(name="tl", bufs=3) as ptl, \
                 tc.tile_pool(name="tpsum", bufs=2, space="PSUM") as ptp, \
                 tc.tile_pool(name="wpsum", bufs=2, space="PSUM") as pwp, \
                 tc.tile_pool(name="wend", bufs=2) as pwe:
                for pi, P in enumerate(passes):
                    twp = P["tw"]
                    is_lo = P["name"] == "lo"
                    if is_lo:
                        b_src = B_d[0 : c.lo_limit, :]
                    else:
                        b_src = B_d[c.lo_limit : c.npad_glob, :]
                    bond_sb = rloc_sb = Ag = Bg = g_sb = None
                    for w in range(c.win):
                        psw = pwp.tile([128, 128], F32, tag="seg")
                        psd = pwp.tile([128, 1], F32, tag="deg")
                        for t in range(twp):
                            tg = w * twp + t
                            sbi, tsb = divmod(tg, c.tps)
                            if tsb == 0:
                                bond_sb = pg.tile([c.bond + 1, c.sb], BF16, tag="bo")
                                nc.sync.dma_start(
                                    out=bond_sb[:],
                                    in_=P["bondT"][:, ds(sbi * c.sb, c.sb)])
                                rloc_sb = pg.tile([128, c.tps], F32, tag="rl")
                                nc.sync.dma_start(
                                    out=rloc_sb[:],
                                    in_=P["rloc"][:, ds(sbi * c.tps, c.tps)])
                                gidx = pg.tile([128, c.sb // 16], I16, tag="gi")
                                nc.sync.dma_start(
                                    out=gidx[:],
                                    in_=P["grow"][:, ds(sbi * (c.sb // 16),
                                                        c.sb // 16)])
                                cidx = pg.tile([128, c.sb // 16], I16, tag="ci")
                                nc.sync.dma_start(
                                    out=cidx[:],
                                    in_=P["col"][:, ds(sbi * (c.sb // 16),
                                                       c.sb // 16)])
                                Ag = pg.tile([128, c.tps, c.d], BF16, tag="ag")
                                nc.gpsimd.dma_gather(
                                    Ag[:], A_d[:, :], gidx[:],
                                    c.sb, c.sb, c.d)
                                Bg = pg.tile([128, c.tps, c.d], BF16, tag="bg")
                                nc.gpsimd.dma_gather(
                                    Bg[:], b_src, cidx[:],
                                    c.sb, c.sb, c.d)
                                psg = pgp.tile([128, c.tps, 1], F32, tag="gp")
                                for u in range(c.tps):
                                    nc.tensor.matmul(
                                        psg[:, u, :],
                                        lhsT=bond_sb[:, ts(u, 128)],
                                        rhs=Wa_sb[:], start=True, stop=True)
                                g_sb = pg.tile([128, c.tps], F32, tag="g")
                                nc.scalar.activation(g_sb[:], psg[:, :, 0], AF.Sigmoid)

                            # --- one tile of 128 edges ---
                            Sg = ptl.tile([128, 128], BF16, tag="sg")
                            nc.vector.tensor_scalar(
                                Sg[:], iotaT_sb[:],
                                rloc_sb[:, ts(tsb, 1)], g_sb[:, ts(tsb, 1)],
                                op0=ALU.is_equal, op1=ALU.mult)
                            pst = ptp.tile([128, 128], F32)
                            nc.tensor.matmul(pst[:],
                                             lhsT=bond_sb[:, ts(tsb, 128)],
                                             rhs=W1c_sb[:], start=True, stop=False)
                            tab = ptl.tile([128, 128], BF16, tag="tab")
                            nc.vector.tensor_tensor(
                                out=tab[:], in0=Ag[:, tsb], in1=Bg[:, tsb],
                                op=ALU.add)
                            nc.tensor.matmul(pst[:], lhsT=iden_sb[:], rhs=tab[:],
                                             start=False, stop=True)
                            rl = ptl.tile([128, 128], BF16, tag="rl2")
                            nc.scalar.activation(rl[:], pst[:], AF.Relu)
                            nc.tensor.matmul(psw[:], lhsT=rl[:], rhs=Sg[:],
                                             start=(t == 0), stop=(t == twp - 1))
                            nc.tensor.matmul(psd[:], lhsT=Sg[:],
                                             rhs=ones_sb[:],
                                             start=(t == 0), stop=(t == twp - 1))

                        # --- window end ---
                        if pi == 0:
                            nc.vector.tensor_copy(
                                out=h1T_sb[:, ts(w, 128)], in_=psw[:])
                            nc.vector.tensor_copy(
                                out=degg_sb[:, ts(w, 1)], in_=psd[:])
                        else:
                            h1t = pwe.tile([128, 128], BF16, tag="h1t")
                            nc.vector.tensor_tensor(
                                out=h1t[:], in0=psw[:],
                                in1=h1T_sb[:, ts(w, 128)], op=ALU.add)
                            dgt = pwe.tile([128, 1], F32, tag="dgt")
                            nc.vector.tensor_tensor(
                                out=dgt[:], in0=psd[:],
                                in1=degg_sb[:, ts(w, 1)], op=ALU.add)
                            pso = pgp.tile([128, c.d], F32, tag="gp")
                            nc.tensor.matmul(pso[:], lhsT=h1t[:], rhs=W2_sb[:],
                                             start=True, stop=True)
                            hw = pwe.tile([128, c.d], F32, tag="hw")
                            nc.sync.dma_start(out=hw[:], in_=hs_d[:, w, :])
                            o1 = pwe.tile([128, c.d], F32, tag="o1")
                            nc.vector.tensor_scalar(
                                o1[:], b2rep_sb[:], dgt[:], None, op0=ALU.mult)
                            o2 = pwe.tile([128, c.d], F32, tag="o2")
                            nc.vector.tensor_tensor(
                                out=o2[:], in0=o1[:], in1=pso[:], op=ALU.add)
                            o3 = pwe.tile([128, c.d], F32, tag="o3")
                            nc.vector.tensor_tensor(
                                out=o3[:], in0=o2[:], in1=hw[:], op=ALU.add)
                            nc.sync.dma_start(out=out_d[:, w, :], in_=o3[:])
    nc.finalize()
    return nc


# --------------------------------------------------------------------------
# Entry point
# --------------------------------------------------------------------------

def assemble(results, cfg=FULL):
    c = cfg
    out = np.empty((c.n, c.d), np.float32)
    for k in range(c.cores):
        o = np.asarray(results[k]["out"])  # [128, win, d]
        o = o.transpose(1, 0, 2).reshape(c.npad_core, c.d)
        nvalid = min(c.npc, c.n - k * c.npc)
        out[k * c.npc : k * c.npc + nvalid] = o[:nvalid]
    return out


def kernel(**inputs):
    cfg = FULL
    in_maps, meta = preprocess(cfg=cfg, **inputs)
    nc = build(meta, cfg=cfg)
    res = run_bass_kernel_spmd(nc, in_maps, list(range(cfg.cores)))
    return assemble(res.results, cfg=cfg)


# revision 16
# speedup vs baseline: 1.9718x; 1.9718x over previous
"""ChemicalGraphConv GNN message-passing kernel for 8 Trainium2 NeuronCores.

Strategy (destination sharding, no collectives):
  - Nodes are split into 8 equal contiguous ranges (6250/core). Each core
    handles the edges whose destination (row) falls in its range and
    produces that slice of the output. No all-reduce needed.
  - Math factorization:  msg_e = relu(h[row]@W1a + h[col]@W1b + bond@W1c + b1)
    gated by g_e = sigmoid(bond@Wa + ba), then @W2 + b2, scatter-add on row.
    Since g_e > 0 and the final @W2 is linear:
       out_r = (sum_e S[e,r] * g_e * relu(t_e)) @ W2 + (sum_e S[e,r] g_e) * b2
    so W2 is applied per 128-row window, not per edge.
  - Per core: precompute A = h_slice@W1a (6272x128) and B = h@W1b (50176x128)
    in bf16 DRAM tables; per edge DMA-gather A[row_local] and B[col].
  - dma_gather indices are int16, so the col gather is done in two passes
    (col < 32768 from B[:32768], col >= 32768 from B[32768:]).
  - Edges are grouped into 49 windows of 128 destination rows; a gated
    one-hot matrix S (built by one DVE tensor_scalar op from iota/rowidx/g)
    does the segment reduction as a matmul accumulating in PSUM.
"""

import math
from dataclasses import dataclass

import ml_dtypes
import numpy as np

import concourse.bass as bass
import concourse.mybir as mybir
import concourse.tile as tile
from concourse import bacc, library_config
from concourse.bass import ds, ts
from concourse.bass_utils import run_bass_kernel_spmd

BF16 = mybir.dt.bfloat16
F32 = mybir.dt.float32
I16 = mybir.dt.int16
NPBF16 = ml_dtypes.bfloat16
NPFP8 = ml_dtypes.float8_e4m3
FP8 = mybir.dt.float8e4

AF = mybir.ActivationFunctionType
ALU = mybir.AluOpType


@dataclass(frozen=True)
class Cfg:
    n: int = 50000          # nodes
    d: int = 128            # feature dim
    bond: int = 64          # bond feature dim
    cores: int = 8
    lo_limit: int = 32768   # int16 gather index split point
    sb: int = 1024          # gather superblock (edges)

    @property
    def npc(self):          # nodes per core (destination range)
        return self.n // self.cores

    @property
    def win(self):          # 128-row windows per core
        return math.ceil(self.npc / 128)

    @property
    def npad_core(self):
        return self.win * 128

    @property
    def npad_glob(self):    # padded global node count (B table rows)
        return ((self.n + 127) // 128) * 128

    @property
    def tps(self):          # tiles (of 128 edges) per superblock
        return self.sb // 128


FULL = Cfg()


# --------------------------------------------------------------------------
# Host-side preprocessing
# --------------------------------------------------------------------------

def _pack16(v):
    # edge i -> [i % 16, i // 16], replicated to 128 partitions (8 q7 cores)
    return np.ascontiguousarray(np.tile(v.reshape(-1, 16).T, (8, 1)))


def _pack128(v):
    # edge i -> [i % 128, i // 128]
    return np.ascontiguousarray(v.reshape(-1, 128).T)


def preprocess(h, edge_index, bond_emb, W1, b1, W2, b2, Wa, ba, cfg=FULL):
    """Returns (in_maps, meta) for the 8 cores."""
    c = cfg
    h = np.asarray(h, np.float32)
    row = np.asarray(edge_index[0]).astype(np.int64)
    col = np.asarray(edge_index[1]).astype(np.int64)
    bond = np.asarray(bond_emb, np.float32)
    W1 = np.asarray(W1, np.float32)
    W2m = np.asarray(W2, np.float32)
    b1 = np.asarray(b1, np.float32)
    b2 = np.asarray(b2, np.float32)
    Wa = np.asarray(Wa, np.float32).reshape(c.bond, 1)
    ba = np.asarray(ba, np.float32).reshape(1)

    core_of = row // c.npc

    # First sweep: per (core, pass) window counts -> global tile budgets
    percore = []
    tw = {"lo": 1, "hi": 1}
    for k in range(c.cores):
        m = core_of == k
        r = (row[m] - k * c.npc).astype(np.int64)
        cc = col[m]
        bb = bond[m]
        lo = cc < c.lo_limit
        entry = {}
        for pname, pm in (("lo", lo), ("hi", ~lo)):
            rp, cp, bp = r[pm], cc[pm], bb[pm]
            w = rp >> 7
            order = np.argsort(w, kind="stable")
            rp, cp, bp, w = rp[order], cp[order], bp[order], w[order]
            counts = np.bincount(w, minlength=c.win).astype(np.int64)
            entry[pname] = (rp, cp, bp, counts)
            if len(rp):
                tw[pname] = max(tw[pname], int(math.ceil(counts.max() / 128)))
        percore.append(entry)

    def stream_len(twp):
        n_tiles = c.win * twp
        n_sb = math.ceil(n_tiles / c.tps)
        return n_sb * c.sb

    L = {p: stream_len(tw[p]) for p in ("lo", "hi")}

    # Constants (shared across cores)
    hT = np.zeros((c.d, c.npad_glob), np.float32)
    hT[:, : c.n] = h.T
    consts = {
        "hT": hT.astype(NPBF16),
        "W1a": np.ascontiguousarray(W1[: c.d]).astype(NPBF16),
        "W1b": np.ascontiguousarray(W1[c.d : 2 * c.d]).astype(NPBF16),
        "W1c": np.vstack([W1[2 * c.d :], b1[None, :]]).astype(NPBF16),
        "Wa": np.vstack([Wa, ba[None, :]]).astype(NPBF16),
        "W2": W2m.astype(NPBF16),
        "b2rep": np.tile(b2[None, :], (128, 1)).astype(np.float32),
        "iden": np.eye(128, dtype=np.float32).astype(NPBF16),
        "iotaT": np.tile(np.arange(128, dtype=np.float32)[None, :], (128, 1)).astype(
            NPBF16
        ),
        "ones": np.ones((128, 1), np.float32).astype(NPBF16),
    }

    in_maps = []
    for k in range(c.cores):
        im = dict(consts)

        # core's h slice, transposed (for A) and window-major f32 (residual)
        hsl = np.zeros((c.npad_core, c.d), np.float32)
        nvalid = min(c.npc, c.n - k * c.npc)
        hsl[:nvalid] = h[k * c.npc : k * c.npc + nvalid]
        im["hTs"] = np.ascontiguousarray(hsl.T).astype(NPBF16)
        im["hs"] = np.ascontiguousarray(
            hsl.reshape(c.win, 128, c.d).transpose(1, 0, 2)
        )

        for pname in ("lo", "hi"):
            rp, cp, bp, counts = percore[k][pname]
            Lp = L[pname]
            twp = tw[pname]
            colv = np.zeros(Lp, np.int64)
            rlocv = np.full(Lp, 999.0, np.float32)
            bT = np.zeros((c.bond + 1, Lp), np.float32)
            bT[c.bond, :] = 1.0
            starts = np.zeros(c.win, np.int64)
            starts[1:] = np.cumsum(counts)[:-1]
            w = rp >> 7
            pos = w * (twp * 128) + (np.arange(len(rp)) - starts[w])
            if pname == "hi":
                colv[pos] = cp - c.lo_limit
            else:
                colv[pos] = cp
            rlocv[pos] = (rp & 127).astype(np.float32)
            bT[: c.bond, pos] = bp.T
            im[f"col_{pname}"] = _pack16(colv.astype(np.int16))
            im[f"rloc_{pname}"] = _pack128(rlocv)
            rl = rlocv.reshape(-1, 128)
            M = (rl[:, :, None] == np.arange(128, dtype=np.float32)[None, None, :])
            im[f"sem_{pname}"] = np.ascontiguousarray(
                M.transpose(1, 0, 2).reshape(128, Lp)).astype(NPFP8)
            im[f"srm_{pname}"] = np.ascontiguousarray(
                M.transpose(2, 0, 1).reshape(128, Lp)).astype(NPFP8)
            im[f"bondT_{pname}"] = bT.astype(NPBF16)
        in_maps.append(im)

    meta = {"tw_lo": tw["lo"], "tw_hi": tw["hi"], "L_lo": L["lo"], "L_hi": L["hi"]}
    return in_maps, meta


# --------------------------------------------------------------------------
# Device program
# --------------------------------------------------------------------------

def build(meta, cfg=FULL):
    c = cfg
    nc = bacc.Bacc("TRN2", target_bir_lowering=False, debug=False)

    def din(name, shape, dt):
        return nc.dram_tensor(name, list(shape), dt, kind="ExternalInput")

    hT_d = din("hT", (c.d, c.npad_glob), BF16)
    hTs_d = din("hTs", (c.d, c.npad_core), BF16)
    hs_d = din("hs", (128, c.win, c.d), F32)
    W1a_d = din("W1a", (c.d, c.d), BF16)
    W1b_d = din("W1b", (c.d, c.d), BF16)
    W1c_d = din("W1c", (c.bond + 1, c.d), BF16)
    Wa_d = din("Wa", (c.bond + 1, 1), BF16)
    W2_d = din("W2", (c.d, c.d), BF16)
    b2rep_d = din("b2rep", (128, c.d), F32)
    iden_d = din("iden", (128, 128), BF16)
    iotaT_d = din("iotaT", (128, 128), BF16)
    ones_d = din("ones", (128, 1), BF16)

    passes = []
    for pname in ("lo", "hi"):
        Lp = meta[f"L_{pname}"]
        passes.append(
            dict(
                name=pname,
                tw=meta[f"tw_{pname}"],
                L=Lp,
                bondT=din(f"bondT_{pname}", (c.bond + 1, Lp), BF16),
                col=din(f"col_{pname}", (128, Lp // 16), I16),
                sem=din(f"sem_{pname}", (128, Lp), FP8),
                srm=din(f"srm_{pname}", (128, Lp), FP8),
            )
        )

    out_d = nc.dram_tensor("out", [128, c.win, c.d], F32, kind="ExternalOutput")
    B_d = nc.dram_tensor("B_tab", [c.npad_glob, c.d], BF16)

    with tile.TileContext(nc) as tc:
        nc.gpsimd.load_library(library_config.mlp)
        with tc.tile_pool(name="const", bufs=1) as cpool:
            W1a_sb = cpool.tile([c.d, c.d], BF16)
            W1b_sb = cpool.tile([c.d, c.d], BF16)
            W1c_sb = cpool.tile([c.bond + 1, c.d], BF16)
            Wa_sb = cpool.tile([c.bond + 1, 1], BF16)
            W2_sb = cpool.tile([c.d, c.d], BF16)
            b2rep_sb = cpool.tile([128, c.d], F32)
            iden_sb = cpool.tile([128, 128], BF16)
            iotaT_sb = cpool.tile([128, 128], BF16)
            ones_sb = cpool.tile([128, 1], BF16)
            for sb_t, dr in (
                (W1a_sb, W1a_d), (W1b_sb, W1b_d), (W1c_sb, W1c_d),
                (Wa_sb, Wa_d), (W2_sb, W2_d), (b2rep_sb, b2rep_d),
                (iden_sb, iden_d), (iotaT_sb, iotaT_d), (ones_sb, ones_d),
            ):
                nc.sync.dma_start(out=sb_t[:], in_=dr[:])

            h1T_sb = cpool.tile([128, c.win * 128], BF16)
            degg_sb = cpool.tile([128, c.win], F32)
            A_sb = cpool.tile([128, c.win, c.d], BF16)

            # ---------------- Phase 1: A and B tables ----------------
            with tc.tile_pool(name="ph1", bufs=3) as p1, \
                 tc.tile_pool(name="ph1p", bufs=8, space="PSUM") as p1p:
                def table_phase(src_d, n_tiles, w_sb, dst_d):
                    for bt in range(math.ceil(n_tiles / 4)):
                        k4 = min(4, n_tiles - bt * 4)
                        hT_t = p1.tile([c.d, 4, 128], BF16, tag="ht")
                        nc.sync.dma_start(
                            out=hT_t[:, :k4, :],
                            in_=src_d[:, ds(bt * 512, k4 * 128)].rearrange(
                                "p (a b) -> p a b", b=128))
                        ob = p1.tile([128, 4, c.d], BF16, tag="ob")
                        for u in range(k4):
                            ps = p1p.tile([128, c.d], F32)
                            nc.tensor.matmul(ps[:], lhsT=hT_t[:, u, :],
                                             rhs=w_sb[:], start=True, stop=True)
                            nc.scalar.activation(ob[:, u, :], ps[:], AF.Copy)
                        nc.sync.dma_start(
                            out=dst_d[ds(bt * 512, k4 * 128), :].rearrange(
                                "(a p) d -> p a d", p=128),
                            in_=ob[:, :k4, :])
                table_phase(hT_d, c.npad_glob // 128, W1b_sb, B_d)
                for w in range(c.win):
                    hT_t = p1.tile([c.d, 128], BF16, tag="ht2")
                    nc.sync.dma_start(out=hT_t[:], in_=hTs_d[:, ts(w, 128)])
                    ps = p1p.tile([128, c.d], F32)
                    nc.tensor.matmul(ps[:], lhsT=hT_t[:], rhs=W1a_sb[:],
                                     start=True, stop=True)
                    nc.scalar.activation(A_sb[:, w, :], ps[:], AF.Copy)

            # ---------------- Phase 2: edge passes ----------------
            with tc.tile_pool(name="gsb", bufs=2) as pg, \
                 tc.tile_pool(name="gpsum", bufs=2, space="PSUM") as pgp, \
                 tc.tile_pool(name="tl", bufs=3) as ptl, \
                 tc.tile_pool(name="tpsum", bufs=2, space="PSUM") as ptp, \
                 tc.tile_pool(name="wpsum", bufs=2, space="PSUM") as pwp, \
                 tc.tile_pool(name="wend", bufs=2) as pwe:
                for pi, P in enumerate(passes):
                    twp = P["tw"]
                    is_lo = P["name"] == "lo"
                    if is_lo:
                        b_src = B_d[0 : c.lo_limit, :]
                    else:
                        b_src = B_d[c.lo_limit : c.npad_glob, :]
                    bond_sb = rloc_sb = Ag = Bg = g_sb = None
                    for w in range(c.win):
                        psw = pwp.tile([128, 128], F32, tag="seg")
                        psd = pwp.tile([128, 1], F32, tag="deg")
                        for t in range(twp):
                            tg = w * twp + t
                            sbi, tsb = divmod(tg, c.tps)
                            if tsb == 0:
                                bond_sb = pg.tile([c.bond + 1, c.sb], BF16, tag="bo")
                                nc.sync.dma_start(
                                    out=bond_sb[:],
                                    in_=P["bondT"][:, ds(sbi * c.sb, c.sb)])
                                sem_sb = pg.tile([128, c.sb], FP8, tag="se")
                                nc.sync.dma_start(
                                    out=sem_sb[:],
                                    in_=P["sem"][:, ds(sbi * c.sb, c.sb)])
                                srm_sb = pg.tile([128, c.sb], FP8, tag="sr")
                                nc.sync.dma_start(
                                    out=srm_sb[:],
                                    in_=P["srm"][:, ds(sbi * c.sb, c.sb)])
                                cidx = pg.tile([128, c.sb // 16], I16, tag="ci")
                                nc.sync.dma_start(
                                    out=cidx[:],
                                    in_=P["col"][:, ds(sbi * (c.sb // 16),
                                                       c.sb // 16)])
                                Bg = pg.tile([128, c.tps, c.d], BF16, tag="bg")
                                nc.gpsimd.dma_gather(
                                    Bg[:], b_src, cidx[:],
                                    c.sb, c.sb, c.d)
                                psg = pgp.tile([128, c.tps, 1], F32, tag="gp")
                                for u in range(c.tps):
                                    nc.tensor.matmul(
                                        psg[:, u, :],
                                        lhsT=bond_sb[:, ts(u, 128)],
                                        rhs=Wa_sb[:], start=True, stop=True)
                                g_sb = pg.tile([128, c.tps], F32, tag="g")
                                nc.scalar.activation(g_sb[:], psg[:, :, 0], AF.Sigmoid)
                                gbf_sb = pg.tile([128, c.tps], BF16, tag="gb")
                                nc.vector.tensor_copy(out=gbf_sb[:], in_=g_sb[:])

                            # --- one tile of 128 edges ---
                            Se = sem_sb[:, ts(tsb, 128)]
                            Srm = srm_sb[:, ts(tsb, 128)]
                            pst = ptp.tile([128, 128], F32)
                            nc.tensor.matmul(pst[:],
                                             lhsT=bond_sb[:, ts(tsb, 128)],
                                             rhs=W1c_sb[:], start=True, stop=False)
                            nc.tensor.matmul(pst[:], lhsT=iden_sb[:],
                                             rhs=Bg[:, tsb],
                                             start=False, stop=False)
                            nc.tensor.matmul(pst[:], lhsT=Srm,
                                             rhs=A_sb[:, w, :],
                                             start=False, stop=True)
                            rl = ptl.tile([128, 128], BF16, tag="rl2")
                            nc.scalar.activation(rl[:], pst[:], AF.Relu,
                                                 scale=g_sb[:, ts(tsb, 1)])
                            nc.tensor.matmul(psw[:], lhsT=rl[:], rhs=Se,
                                             start=(t == 0), stop=(t == twp - 1))
                            nc.tensor.matmul(psd[:], lhsT=Se,
                                             rhs=gbf_sb[:, ts(tsb, 1)],
                                             start=(t == 0), stop=(t == twp - 1))

                        # --- window end ---
                        if pi == 0:
                            nc.vector.tensor_copy(
                                out=h1T_sb[:, ts(w, 128)], in_=psw[:])
                            nc.vector.tensor_copy(
                                out=degg_sb[:, ts(w, 1)], in_=psd[:])
                        else:
                            h1t = pwe.tile([128, 128], BF16, tag="h1t")
                            nc.vector.tensor_tensor(
                                out=h1t[:], in0=psw[:],
                                in1=h1T_sb[:, ts(w, 128)], op=ALU.add)
                            dgt = pwe.tile([128, 1], F32, tag="dgt")
                            nc.vector.tensor_tensor(
                                out=dgt[:], in0=psd[:],
                                in1=degg_sb[:, ts(w, 1)], op=ALU.add)
                            pso = pgp.tile([128, c.d], F32, tag="gp")
                            nc.tensor.matmul(pso[:], lhsT=h1t[:], rhs=W2_sb[:],
                                             start=True, stop=True)
                            hw = pwe.tile([128, c.d], F32, tag="hw")
                            nc.sync.dma_start(out=hw[:], in_=hs_d[:, w, :])
                            o1 = pwe.tile([128, c.d], F32, tag="o1")
                            nc.vector.tensor_scalar(
                                o1[:], b2rep_sb[:], dgt[:], None, op0=ALU.mult)
                            o2 = pwe.tile([128, c.d], F32, tag="o2")
                            nc.vector.tensor_tensor(
                                out=o2[:], in0=o1[:], in1=pso[:], op=ALU.add)
                            o3 = pwe.tile([128, c.d], F32, tag="o3")
                            nc.vector.tensor_tensor(
                                out=o3[:], in0=o2[:], in1=hw[:], op=ALU.add)
                            nc.sync.dma_start(out=out_d[:, w, :], in_=o3[:])
    nc.finalize()
    return nc


# --------------------------------------------------------------------------
# Entry point
# --------------------------------------------------------------------------

def assemble(results, cfg=FULL):
    c = cfg
    out = np.empty((c.n, c.d), np.float32)
    for k in range(c.cores):
        o = np.asarray(results[k]["out"])  # [128, win, d]
        o = o.transpose(1, 0, 2).reshape(c.npad_core, c.d)
        nvalid = min(c.npc, c.n - k * c.npc)
        out[k * c.npc : k * c.npc + nvalid] = o[:nvalid]
    return out


def kernel(**inputs):
    cfg = FULL
    in_maps, meta = preprocess(cfg=cfg, **inputs)
    nc = build(meta, cfg=cfg)
    res = run_bass_kernel_spmd(nc, in_maps, list(range(cfg.cores)))
    return assemble(res.results, cfg=cfg)
